# revision 1
# baseline (speedup 1.0000x reference)
"""Trainium2 Bass kernel: inclusive cumsum along L for X (4, 8192, 32, 32) f32.

Strategy (8 NeuronCores, SPMD):
  - View X as (B=4, L=8192, C=1024) with C = D*N flattened. The 4096 scan
    rows (b, c) are independent; shard them 8 ways: core i gets b = i//2 and
    the c-half h = i%2, i.e. a (8192, 512) slab whose DRAM rows are 2 KiB
    contiguous.
  - Per core: stream 512-long L superblocks (1 MiB batched DMAs). Each
    (128 l, 128 c) sub-tile is PE-transposed into PSUM banks laid out as
    (128 c, 512 l). The VectorE tensor_tensor_scan runs the cumsum along the
    free (l) dim, chaining superblocks via the `initial` operand (carry =
    last column of the previous scan output). Scan outputs are PE-transposed
    back to natural (l, c) layout in PSUM, copied to SBUF by ScalarE, and
    DMA'd out as 1 MiB transfers.
  - Engine budget per core (measured): DMA ~94-105 us saturated at the
    ~358 GB/s HBM-per-core limit (the bottleneck), DVE scans ~81 us, PE
    transposes ~70 us, ACT copies ~36 us, plus ~8 us NEFF preamble and
    ~9 us Tile exit barrier. In-DMAs rotate across the Sync/GPSIMD/Scalar
    issue paths and out-DMAs alternate GPSIMD/Sync for DMA-queue diversity.
    Measured ~112 us mean / ~115 us slowest-core on 8 NC-v3 cores.
"""

import numpy as np
from contextlib import ExitStack

import concourse.bass as bass
import concourse.tile as tile
from concourse import bacc, masks, mybir
from concourse.bass_utils import run_bass_kernel_spmd

N_CORES = 8
B, L, D, N = 4, 8192, 32, 32
C_FULL = D * N          # 1024 columns per batch entry
C = C_FULL // 2         # 512 columns per core
P = 128                 # partitions
SUPER = 512             # L elems per superblock
N_SUPER = L // SUPER    # 16
BLKS = SUPER // P       # 4 L-blocks per superblock
CGRP = C // P           # 4 column groups per core

_CACHE = {}


def _build_program():
    f32 = mybir.dt.float32
    nc = bacc.Bacc(
        trn_type="TRN2", debug=False, num_devices=N_CORES, num_swdge_queues=2
    )
    x = nc.dram_tensor("x", [L, C], f32, kind="ExternalInput").ap()
    y = nc.dram_tensor("y", [L, C], f32, kind="ExternalOutput").ap()

    with tile.TileContext(nc) as tc, ExitStack() as ctx:
        const_pool = ctx.enter_context(tc.tile_pool(name="const", bufs=1))
        xin_pool = ctx.enter_context(tc.tile_pool(name="xin", bufs=6))
        scano_pool = ctx.enter_context(tc.tile_pool(name="scano", bufs=2))
        yout_pool = ctx.enter_context(tc.tile_pool(name="yout", bufs=6))
        inps_pool = ctx.enter_context(tc.tile_pool(name="inps", bufs=4, space="PSUM"))
        outps_pool = ctx.enter_context(tc.tile_pool(name="outps", bufs=2, space="PSUM"))

        ident = const_pool.tile([P, P], f32, name="ident")
        masks.make_identity(nc, ident[:])
        zeros = const_pool.tile([P, SUPER], f32, name="zeros")
        nc.gpsimd.memset(zeros[:], 0.0)

        prev = [None] * CGRP
        for t in range(N_SUPER):
            # ---- load the whole superblock with one 1 MiB DMA ----
            # DRAM rows l = t*512 + ks*128 + p; element order [p][ks][c] on
            # both sides so the 3D APs pair up.
            xt = xin_pool.tile([P, BLKS * C], f32, name="xt", tag="xt", bufs=6)
            src = x[t * SUPER : (t + 1) * SUPER, :].rearrange(
                "(ks p) c -> p ks c", p=P
            )
            dst = xt[:].rearrange("p (ks c) -> p ks c", ks=BLKS)
            in_eng = (nc.sync, nc.gpsimd, nc.scalar)[t % 3]
            in_eng.dma_start(out=dst, in_=src)

            # ---- transpose into (c, l) PSUM banks; scan along l ----
            souts = []
            for j in range(CGRP):
                ib = inps_pool.tile([P, SUPER], f32, name="ib", tag="ib", bufs=4)
                for ks in range(BLKS):
                    nc.tensor.transpose(
                        ib[:, ks * P : (ks + 1) * P],
                        xt[:, ks * C + j * P : ks * C + (j + 1) * P],
                        ident[:],
                    )
                so = scano_pool.tile(
                    [P, SUPER], f32, name=f"so{j}", tag=f"so{j}", bufs=2
                )
                init = 0.0 if t == 0 else prev[j][:, SUPER - 1 : SUPER]
                nc.vector.tensor_tensor_scan(
                    so[:], ib[:], zeros[:], init,
                    mybir.AluOpType.add, mybir.AluOpType.add,
                )
                souts.append(so)
            prev = souts

            # ---- transpose back to (l, c); ScalarE copies PSUM->SBUF ----
            yt = yout_pool.tile([P, BLKS * C], f32, name="yt", tag="yt", bufs=6)
            for half in range(2):
                ob = outps_pool.tile([P, 2 * C], f32, name="ob", tag="ob", bufs=2)
                for i2 in range(2):
                    i = half * 2 + i2
                    for j in range(CGRP):
                        nc.tensor.transpose(
                            ob[:, i2 * C + j * P : i2 * C + (j + 1) * P],
                            souts[j][:, i * P : (i + 1) * P],
                            ident[:],
                        )
                nc.scalar.copy(yt[:, half * 2 * C : (half + 1) * 2 * C], ob[:])

            ydst = y[t * SUPER : (t + 1) * SUPER, :].rearrange(
                "(ks p) c -> p ks c", p=P
            )
            ysrc = yt[:].rearrange("p (ks c) -> p ks c", ks=BLKS)
            out_eng = nc.gpsimd if t % 2 == 0 else nc.sync
            out_eng.dma_start(out=ydst, in_=ysrc)

    nc.compile()
    return nc


def _get_program():
    if "nc" not in _CACHE:
        _CACHE["nc"] = _build_program()
    return _CACHE["nc"]


def _shard(X):
    """(4, 8192, 32, 32) -> 8 contiguous (8192, 512) slabs."""
    Xv = X.reshape(B, L, C_FULL)
    shards = []
    for i in range(N_CORES):
        b, h = i // 2, i % 2
        shards.append(np.ascontiguousarray(Xv[b, :, h * C : (h + 1) * C]))
    return shards


def _unshard(parts):
    out = np.empty((B, L, C_FULL), dtype=np.float32)
    for i in range(N_CORES):
        b, h = i // 2, i % 2
        out[b, :, h * C : (h + 1) * C] = parts[i]
    return out.reshape(B, L, D, N)


def kernel(X_in, _trace=False, _tmpdir=None, _trace_cores=None):
    X = np.asarray(X_in, dtype=np.float32)
    assert X.shape == (B, L, D, N), X.shape
    nc = _get_program()
    in_maps = [{"x": s} for s in _shard(X)]
    kwargs = {}
    if _trace:
        kwargs = dict(
            trace=True,
            tmpdir=_tmpdir,
            trace_cores=_trace_cores or list(range(N_CORES)),
        )
    res = run_bass_kernel_spmd(nc, in_maps, core_ids=list(range(N_CORES)), **kwargs)
    out = _unshard([res.results[i]["y"] for i in range(N_CORES)])
    kernel.last_results = res
    return out



# revision 2
# speedup vs baseline: 1.1412x; 1.1412x over previous
"""Trainium2 Bass kernel: inclusive cumsum along L for X (4, 8192, 32, 32) f32.

Strategy (8 NeuronCores, SPMD), v1 "all-matmul fp16":
  - View X as (B=4, L=8192, C=1024), shard 8 ways: core i gets b = i//2,
    c-half h = i%2 -> a (8192, 512) slab. Host converts to fp16 (the
    rel-err budget is 2e-2 of max|cumsum| ~ 400, fp16 error ~1e-3), which
    halves HBM traffic: 8.4 MB in + 8.4 MB out per core ~ 47 us at the
    ~358 GB/s per-core HBM limit.
  - No DVE scan, no transposes. The cumsum is done by TensorE matmuls in
    the natural (l-partition, c-free) layout:
      Phase A: 8x 1MiB in-DMAs; per 128-l block t, a one-hot-weight
        matmul (lhsT[k,m]=[m==t]) accumulates the block's column totals
        into row t of one PSUM bank -> totals[64, 512]; one DVE copy
        downcasts to fp16 SBUF.
      Phase B: per block t, UT-ones matmul (lhsT[k,m]=[k<=m]) gives the
        intra-block inclusive prefix; a prefix-mask matmul
        (lhsT[k,m]=[k<t]) over totals_sb accumulates the inter-block
        carry into the same PSUM bank. ACT/DVE alternate on the
        PSUM->SBUF fp16 downcast copy; 8 blocks batch into 1MiB out-DMAs.
  - Constant weights (UT, one-hots, prefix masks; ~2 MB) are generated on
    the host and DMA'd in as extra inputs.
"""

import numpy as np
from contextlib import ExitStack

import concourse.bass as bass
import concourse.tile as tile
from concourse import bacc, mybir
from concourse.bass_utils import run_bass_kernel_spmd

N_CORES = 8
B, L, D, N = 4, 8192, 32, 32
C_FULL = D * N          # 1024 columns per batch entry
C = C_FULL // 2         # 512 columns per core
P = 128                 # partitions / l-block size
NBLK = L // P           # 64 l-blocks per core
KS = 8                  # l-blocks per DMA superchunk
NDMA = NBLK // KS       # 8 DMAs each way

_CACHE = {}


def _host_consts():
    ut = np.triu(np.ones((P, P), dtype=np.float16))          # [k,m]=k<=m
    oh = np.zeros((P, NBLK * NBLK), dtype=np.float16)        # oh_t: [k, m]=[m==t]
    for t in range(NBLK):
        oh[:, t * NBLK + t] = 1.0
    pm = np.zeros((NBLK, NBLK * P), dtype=np.float16)        # pm_t: [k, m]=[k<t]
    for t in range(NBLK):
        pm[:t, t * P : (t + 1) * P] = 1.0
    return ut, oh, pm


def _build_program():
    f16 = mybir.dt.float16
    f32 = mybir.dt.float32
    nc = bacc.Bacc(
        trn_type="TRN2", debug=False, num_devices=N_CORES, num_swdge_queues=2
    )
    x = nc.dram_tensor("x", [L, C], f16, kind="ExternalInput").ap()
    y = nc.dram_tensor("y", [L, C], f16, kind="ExternalOutput").ap()
    utd = nc.dram_tensor("ut", [P, P], f16, kind="ExternalInput").ap()
    ohd = nc.dram_tensor("oh", [P, NBLK * NBLK], f16, kind="ExternalInput").ap()
    pmd = nc.dram_tensor("pm", [NBLK, NBLK * P], f16, kind="ExternalInput").ap()

    with tile.TileContext(nc) as tc, ExitStack() as ctx:
        const_pool = ctx.enter_context(tc.tile_pool(name="const", bufs=1))
        x_pool = ctx.enter_context(tc.tile_pool(name="xin", bufs=1))
        y_pool = ctx.enter_context(tc.tile_pool(name="yout", bufs=3))
        tot_psum = ctx.enter_context(tc.tile_pool(name="totp", bufs=1, space="PSUM"))
        main_psum = ctx.enter_context(tc.tile_pool(name="mainp", bufs=6, space="PSUM"))

        ut_sb = const_pool.tile([P, P], f16, name="ut")
        oh_sb = const_pool.tile([P, NBLK * NBLK], f16, name="oh")
        pm_sb = const_pool.tile([NBLK, NBLK * P], f16, name="pm")
        totals_sb = const_pool.tile([NBLK, C], f16, name="totals")
        xall = x_pool.tile([P, NBLK * C], f16, name="xall")

        nc.sync.dma_start(out=ut_sb[:], in_=utd[:, :])
        nc.sync.dma_start(out=oh_sb[:], in_=ohd[:, :])
        nc.sync.dma_start(out=pm_sb[:], in_=pmd[:, :])

        # ---- Phase A: stream input, accumulate per-block column totals ----
        for d in range(NDMA):
            src = x[d * KS * P : (d + 1) * KS * P, :].rearrange(
                "(ks p) c -> p ks c", p=P
            )
            dst = xall[:, d * KS * C : (d + 1) * KS * C].rearrange(
                "p (ks c) -> p ks c", ks=KS
            )
            eng = nc.sync if d % 2 == 0 else nc.gpsimd
            eng.dma_start(out=dst, in_=src)

        totalsP = tot_psum.tile([NBLK, C], f32, name="totalsP")
        for t in range(NBLK):
            nc.tensor.matmul(
                totalsP[:],
                oh_sb[:, t * NBLK : (t + 1) * NBLK],
                xall[:, t * C : (t + 1) * C],
                start=(t == 0),
                stop=(t == NBLK - 1),
            )
        nc.vector.tensor_copy(totals_sb[:], totalsP[:])

        # ---- Phase B: intra-block prefix + carry, downcast, stream out ----
        yb = None
        for t in range(NBLK):
            bank = main_psum.tile([P, C], f32, name="bank", tag="bank", bufs=6)
            nc.tensor.matmul(
                bank[:],
                ut_sb[:],
                xall[:, t * C : (t + 1) * C],
                start=True,
                stop=(t == 0),
            )
            if t > 0:
                nc.tensor.matmul(
                    bank[:],
                    pm_sb[:, t * P : (t + 1) * P],
                    totals_sb[:],
                    start=False,
                    stop=True,
                )
            j = t % KS
            if j == 0:
                yb = y_pool.tile([P, KS * C], f16, name="yb", tag="yb", bufs=3)
            if t % 2 == 0:
                nc.scalar.copy(yb[:, j * C : (j + 1) * C], bank[:])
            else:
                nc.vector.tensor_copy(yb[:, j * C : (j + 1) * C], bank[:])
            if j == KS - 1:
                d = t // KS
                ydst = y[d * KS * P : (d + 1) * KS * P, :].rearrange(
                    "(ks p) c -> p ks c", p=P
                )
                ysrc = yb[:].rearrange("p (ks c) -> p ks c", ks=KS)
                eng = nc.gpsimd if d % 2 == 0 else nc.sync
                eng.dma_start(out=ydst, in_=ysrc)

    nc.compile()
    return nc


def _get_program():
    if "nc" not in _CACHE:
        _CACHE["nc"] = _build_program()
    return _CACHE["nc"]


def _shard(X):
    """(4, 8192, 32, 32) f32 -> 8 contiguous (8192, 512) fp16 slabs."""
    Xv = X.reshape(B, L, C_FULL)
    shards = []
    for i in range(N_CORES):
        b, h = i // 2, i % 2
        shards.append(
            np.ascontiguousarray(Xv[b, :, h * C : (h + 1) * C], dtype=np.float16)
        )
    return shards


def _unshard(parts):
    out = np.empty((B, L, C_FULL), dtype=np.float32)
    for i in range(N_CORES):
        b, h = i // 2, i % 2
        out[b, :, h * C : (h + 1) * C] = parts[i].astype(np.float32)
    return out.reshape(B, L, D, N)


def kernel(X_in, _trace=False, _tmpdir=None, _trace_cores=None):
    X = np.asarray(X_in, dtype=np.float32)
    assert X.shape == (B, L, D, N), X.shape
    nc = _get_program()
    ut, oh, pm = _host_consts()
    in_maps = [{"x": s, "ut": ut, "oh": oh, "pm": pm} for s in _shard(X)]
    kwargs = {}
    if _trace:
        kwargs = dict(
            trace=True,
            tmpdir=_tmpdir,
            trace_cores=_trace_cores or list(range(N_CORES)),
        )
    res = run_bass_kernel_spmd(nc, in_maps, core_ids=list(range(N_CORES)), **kwargs)
    out = _unshard([res.results[i]["y"] for i in range(N_CORES)])
    kernel.last_results = res
    return out


# revision 5
# speedup vs baseline: 1.4084x; 1.2342x over previous
"""Trainium2 Bass kernel: inclusive cumsum along L for X (4, 8192, 32, 32) f32.

Strategy (8 NeuronCores, SPMD), v3 "pipelined all-matmul fp16":
  - View X as (B=4, L=8192, C=1024), shard 8 ways: core i gets b = i//2,
    c-half h = i%2 -> a (8192, 512) slab, converted to fp16 on the host
    (error budget is 2e-2 of max|cumsum|~400; fp16 gives ~5e-4). HBM
    traffic per core: 8.4 MB in + 8.4 MB out ~ 47 us at the ~358 GB/s
    per-core limit. The host pre-swizzles each slab to (g, p, ks, c)
    order so every DMA is a fully contiguous 1 MiB block (128
    descriptors x 8 KiB), and un-swizzles the output.
  - Compute is all TensorE matmuls in natural (l-partition, c-free)
    layout, pipelined in 8 groups of 8 l-blocks (block = 128 l):
      per group g: 1 MiB in-DMA (sync HWDGE ring, FIFO order) ->
      8 one-hot matmuls accumulate per-block column totals into a
      persistent [64,512] PSUM bank -> ACT full-copy to totals_sb ->
      one prefix-mask matmul forms the 8 block carries [8,512] -> ACT
      copy -> one SWDGE accumulate-DMA adds carry_t into l-row 0 of
      block t (then cumsum = UT @ [x0+carry; x1; ...]) -> 8 back-to-back
      UT-ones matmuls (shared weights) -> ACT/DVE alternate PSUM->SBUF
      fp16 copies -> 1 MiB out-DMA (gpsimd).
"""

import numpy as np
from contextlib import ExitStack

import concourse.bass as bass
import concourse.tile as tile
from concourse import bacc, mybir
from concourse.bass_utils import run_bass_kernel_spmd

N_CORES = 8
B, L, D, N = 4, 8192, 32, 32
C_FULL = D * N          # 1024 columns per batch entry
C = C_FULL // 2         # 512 columns per core
P = 128                 # partitions / l-block size
NBLK = L // P           # 64 l-blocks per core
KS = 8                  # l-blocks per group (= per DMA)
NG = NBLK // KS         # 8 groups

_CACHE = {}


def _host_consts():
    ut = np.triu(np.ones((P, P), dtype=np.float16))     # [k,m] = k<=m
    ohg = np.zeros((P, NBLK * NBLK), dtype=np.float16)  # ohg_t: [k,m] = [m==t]
    for t in range(NBLK):
        ohg[:, t * NBLK + t] = 1.0
    pm8 = np.zeros((NBLK, NG * KS), dtype=np.float16)   # pm8_g col j: [k<8g+j]
    for g in range(NG):
        for j in range(KS):
            pm8[: g * KS + j, g * KS + j] = 1.0
    return ut, ohg, pm8


def _build_program():
    f16 = mybir.dt.float16
    f32 = mybir.dt.float32
    nc = bacc.Bacc(
        trn_type="TRN2", debug=False, num_devices=N_CORES, num_swdge_queues=2
    )
    x = nc.dram_tensor("x", [NG, P, KS * C], f16, kind="ExternalInput").ap()
    y = nc.dram_tensor("y", [NG, P, KS * C], f16, kind="ExternalOutput").ap()
    utd = nc.dram_tensor("ut", [P, P], f16, kind="ExternalInput").ap()
    ohd = nc.dram_tensor("ohg", [P, NBLK * NBLK], f16, kind="ExternalInput").ap()
    pmd = nc.dram_tensor("pm8", [NBLK, NG * KS], f16, kind="ExternalInput").ap()

    with tile.TileContext(nc) as tc, ExitStack() as ctx:
        const_pool = ctx.enter_context(tc.tile_pool(name="const", bufs=1))
        x_pool = ctx.enter_context(tc.tile_pool(name="xin", bufs=1))
        y_pool = ctx.enter_context(tc.tile_pool(name="yout", bufs=3))
        stage_pool = ctx.enter_context(tc.tile_pool(name="stage", bufs=2))
        tot_psum = ctx.enter_context(tc.tile_pool(name="totp", bufs=1, space="PSUM"))
        car_psum = ctx.enter_context(tc.tile_pool(name="carp", bufs=2, space="PSUM"))
        main_psum = ctx.enter_context(tc.tile_pool(name="mainp", bufs=5, space="PSUM"))

        ut_sb = const_pool.tile([P, P], f16, name="ut")
        oh_sb = const_pool.tile([P, NBLK * NBLK], f16, name="ohg")
        pm_sb = const_pool.tile([NBLK, NG * KS], f16, name="pm8")
        totals_sb = const_pool.tile([NBLK, C], f16, name="totals")
        xall = x_pool.tile([P, NBLK * C], f16, name="xall")

        nc.sync.dma_start(out=ut_sb[:], in_=utd[:, :])
        nc.sync.dma_start(out=pm_sb[:], in_=pmd[:, :])
        nc.sync.dma_start(out=oh_sb[:], in_=ohd[:, :])

        # All in-DMAs up front on the sync HWDGE ring: FIFO order means
        # group g's data lands before group g+1's, so compute pipelines.
        for g in range(NG):
            xg = xall[:, g * KS * C : (g + 1) * KS * C]
            nc.sync.dma_start(out=xg, in_=x[g])

        totP = tot_psum.tile([NBLK, C], f32, name="totP")
        for g in range(NG):
            # ---- per-block column totals into rows 8g..8g+7 of totP ----
            for j in range(KS):
                t = g * KS + j
                nc.tensor.matmul(
                    totP[:],
                    oh_sb[:, t * NBLK : (t + 1) * NBLK],
                    xall[:, t * C : (t + 1) * C],
                    start=(t == 0),
                    stop=(t == NBLK - 1),
                )
            # full-bank copy: rows >= 8g+8 are not yet accumulated, but
            # the carry matmul below only reads rows < 8g+8.
            nc.scalar.copy(totals_sb[:], totP[:])

            # ---- the 8 block carries of this group in one matmul ----
            kext = g * KS + KS
            carP = car_psum.tile([KS, C], f32, name="carP", tag="carP", bufs=2)
            nc.tensor.matmul(
                carP[:],
                pm_sb[:kext, g * KS : (g + 1) * KS],
                totals_sb[:kext, :],
                start=True,
                stop=True,
            )
            car_stage = stage_pool.tile([KS, C], f16, name="cst", tag="cst", bufs=2)
            nc.scalar.copy(car_stage[:], carP[:])

            # ---- fold carry_t into l-row 0 of block t via accum-DMA ----
            dst = xall[0:1, g * KS * C : (g + 1) * KS * C].rearrange(
                "p (j c) -> p j c", j=KS
            )
            nc.gpsimd.dma_start(
                out=dst, in_=car_stage[:], accum_op=mybir.AluOpType.add
            )

            # ---- 8 back-to-back UT matmuls; copy out; stream to HBM ----
            yb = y_pool.tile([P, KS * C], f16, name="yb", tag="yb", bufs=3)
            for j in range(KS):
                t = g * KS + j
                bank = main_psum.tile([P, C], f32, name="bank", tag="bank", bufs=5)
                nc.tensor.matmul(
                    bank[:],
                    ut_sb[:],
                    xall[:, t * C : (t + 1) * C],
                    start=True,
                    stop=True,
                )
                if j % 2 == 0:
                    nc.scalar.copy(yb[:, j * C : (j + 1) * C], bank[:])
                else:
                    nc.vector.tensor_copy(yb[:, j * C : (j + 1) * C], bank[:])
            nc.gpsimd.dma_start(out=y[g], in_=yb[:])

    nc.compile()
    return nc


def _get_program():
    if "nc" not in _CACHE:
        _CACHE["nc"] = _build_program()
    return _CACHE["nc"]


def _shard(X):
    """(4, 8192, 32, 32) f32 -> 8 fp16 slabs swizzled to (g, p, ks, c)."""
    Xv = X.reshape(B, L, C_FULL)
    shards = []
    for i in range(N_CORES):
        b, h = i // 2, i % 2
        s = Xv[b, :, h * C : (h + 1) * C].astype(np.float16)    # (8192, 512)
        s = s.reshape(NG, KS, P, C).transpose(0, 2, 1, 3)       # (g, p, ks, c)
        shards.append(np.ascontiguousarray(s.reshape(NG, P, KS * C)))
    return shards


def _unshard(parts):
    out = np.empty((B, L, C_FULL), dtype=np.float32)
    for i in range(N_CORES):
        b, h = i // 2, i % 2
        p = parts[i].reshape(NG, P, KS, C).transpose(0, 2, 1, 3)  # (g, ks, p, c)
        out[b, :, h * C : (h + 1) * C] = p.reshape(L, C).astype(np.float32)
    return out.reshape(B, L, D, N)


def kernel(X_in, _trace=False, _tmpdir=None, _trace_cores=None):
    X = np.asarray(X_in, dtype=np.float32)
    assert X.shape == (B, L, D, N), X.shape
    nc = _get_program()
    ut, ohg, pm8 = _host_consts()
    in_maps = [{"x": s, "ut": ut, "ohg": ohg, "pm8": pm8} for s in _shard(X)]
    kwargs = {}
    if _trace:
        kwargs = dict(
            trace=True,
            tmpdir=_tmpdir,
            trace_cores=_trace_cores or list(range(N_CORES)),
        )
    res = run_bass_kernel_spmd(nc, in_maps, core_ids=list(range(N_CORES)), **kwargs)
    out = _unshard([res.results[i]["y"] for i in range(N_CORES)])
    kernel.last_results = res
    return out


# revision 7
# speedup vs baseline: 1.4984x; 1.0639x over previous
"""Trainium2 Bass kernel: inclusive cumsum along L for X (4, 8192, 32, 32) f32.

Strategy (8 NeuronCores, SPMD), v4 "pipelined all-matmul fp16":
  - View X as (B=4, L=8192, C=1024), shard 8 ways: core i gets b = i//2,
    c-half h = i%2 -> a (8192, 512) slab, converted to fp16 on the host
    (error budget is 2e-2 of max|cumsum|~400; fp16 gives ~1e-3). HBM
    traffic per core: 8.4 MB in + 8.4 MB out ~ 47 us at the ~358 GB/s
    per-core limit. The host pre-swizzles each slab to (g, p, ks, c)
    order so every DMA is a fully contiguous block, and un-swizzles the
    output.
  - Compute is all TensorE matmuls in natural (l-partition, c-free)
    layout (back-to-back matmuls stream at ~216 ns for N=512), pipelined
    in 8 groups of 8 l-blocks (block = 128 l) with one group of
    software-pipeline skew so the carry chain never stalls the PE FIFO:
      group g: 1 MiB in-DMA (sync HWDGE ring, FIFO order) ->
      8 one-hot matmuls (sliding-window constant) accumulate per-block
      column totals into a persistent [64,512] PSUM bank -> ACT
      full-bank copy to totals_sb -> one triangle-mask matmul forms the
      8 block carries [8,512] -> ACT copy -> SWDGE accumulate-DMA adds
      carry_t into l-row 0 of block t; one group later: 8 back-to-back
      UT-ones matmuls (cumsum = UT @ [x0+carry; x1; ...]) -> ACT/DVE
      alternate PSUM->SBUF fp16 copies -> 512 KiB out-DMAs (gpsimd).
"""

import numpy as np
from contextlib import ExitStack

import concourse.bass as bass
import concourse.tile as tile
from concourse import bacc, mybir
from concourse.bass_utils import run_bass_kernel_spmd

N_CORES = 8
B, L, D, N = 4, 8192, 32, 32
C_FULL = D * N          # 1024 columns per batch entry
C = C_FULL // 2         # 512 columns per core
P = 128                 # partitions / l-block size
NBLK = L // P           # 64 l-blocks per core
KS = 8                  # l-blocks per group (= per in-DMA)
NG = NBLK // KS         # 8 groups

_CACHE = {}


def _host_consts():
    ut = np.triu(np.ones((P, P), dtype=np.float16))       # [k,m] = k<=m
    # Sliding one-hot: ohw[:, 63-j : 127-j] has ones exactly in column j.
    ohw = np.zeros((P, 2 * NBLK - 1), dtype=np.float16)
    ohw[:, NBLK - 1] = 1.0
    # Strict lower triangle: pmw[k, c] = [k < c]; slice [0:8g+8, 8g:8g+8]
    # has column j = [k < 8g+j].
    pmw = np.zeros((NBLK, NBLK + KS), dtype=np.float16)
    for c in range(NBLK + KS):
        pmw[: min(c, NBLK), c] = 1.0
    return ut, ohw, pmw


def _build_program():
    f16 = mybir.dt.float16
    f32 = mybir.dt.float32
    nc = bacc.Bacc(
        trn_type="TRN2", debug=False, num_devices=N_CORES, num_swdge_queues=2
    )
    x = nc.dram_tensor("x", [NG, P, KS * C], f16, kind="ExternalInput").ap()
    y = nc.dram_tensor("y", [NG, P, KS * C], f16, kind="ExternalOutput").ap()
    utd = nc.dram_tensor("ut", [P, P], f16, kind="ExternalInput").ap()
    ohd = nc.dram_tensor("ohw", [P, 2 * NBLK - 1], f16, kind="ExternalInput").ap()
    pmd = nc.dram_tensor("pmw", [NBLK, NBLK + KS], f16, kind="ExternalInput").ap()

    with tile.TileContext(nc) as tc, ExitStack() as ctx:
        const_pool = ctx.enter_context(tc.tile_pool(name="const", bufs=1))
        x_pool = ctx.enter_context(tc.tile_pool(name="xin", bufs=1))
        y_pool = ctx.enter_context(tc.tile_pool(name="yout", bufs=3))
        stage_pool = ctx.enter_context(tc.tile_pool(name="stage", bufs=2))
        tot_psum = ctx.enter_context(tc.tile_pool(name="totp", bufs=1, space="PSUM"))
        car_psum = ctx.enter_context(tc.tile_pool(name="carp", bufs=2, space="PSUM"))
        main_psum = ctx.enter_context(tc.tile_pool(name="mainp", bufs=5, space="PSUM"))

        ut_sb = const_pool.tile([P, P], f16, name="ut")
        oh_sb = const_pool.tile([P, 2 * NBLK - 1], f16, name="ohw")
        pm_sb = const_pool.tile([NBLK, NBLK + KS], f16, name="pmw")
        totals_sb = const_pool.tile([NBLK, C], f16, name="totals")
        xall = x_pool.tile([P, NBLK * C], f16, name="xall")

        nc.sync.dma_start(out=ut_sb[:], in_=utd[:, :])
        nc.sync.dma_start(out=pm_sb[:], in_=pmd[:, :])
        nc.sync.dma_start(out=oh_sb[:], in_=ohd[:, :])

        # All in-DMAs up front on the sync HWDGE ring: FIFO order means
        # group g's data lands before group g+1's, so compute pipelines.
        for g in range(NG):
            xg = xall[:, g * KS * C : (g + 1) * KS * C]
            nc.sync.dma_start(out=xg, in_=x[g])

        totP = tot_psum.tile([NBLK, C], f32, name="totP")

        def carry_stage(g):
            """Totals + carries for group g; fold into xall row 0."""
            for j in range(KS):
                t = g * KS + j
                nc.tensor.matmul(
                    totP[:],
                    oh_sb[:, NBLK - 1 - t : 2 * NBLK - 1 - t],
                    xall[:, t * C : (t + 1) * C],
                    start=(t == 0),
                    stop=(t == NBLK - 1),
                )
            # full-bank copy: rows >= 8g+8 not yet accumulated, but the
            # carry matmul only reads rows < 8g+8.
            nc.scalar.copy(totals_sb[:], totP[:])
            kext = g * KS + KS
            carP = car_psum.tile([KS, C], f32, name="carP", tag="carP", bufs=2)
            nc.tensor.matmul(
                carP[:],
                pm_sb[:kext, g * KS : (g + 1) * KS],
                totals_sb[:kext, :],
                start=True,
                stop=True,
            )
            car_stage = stage_pool.tile([KS, C], f16, name="cst", tag="cst", bufs=2)
            nc.scalar.copy(car_stage[:], carP[:])
            dst = xall[0:1, g * KS * C : (g + 1) * KS * C].rearrange(
                "p (j c) -> p j c", j=KS
            )
            nc.gpsimd.dma_start(
                out=dst, in_=car_stage[:], accum_op=mybir.AluOpType.add
            )

        def ut_stage(g):
            """8 back-to-back UT matmuls; copy out; stream to HBM."""
            yb = y_pool.tile([P, KS * C], f16, name="yb", tag="yb", bufs=3)
            for j in range(KS):
                t = g * KS + j
                bank = main_psum.tile([P, C], f32, name="bank", tag="bank", bufs=5)
                nc.tensor.matmul(
                    bank[:],
                    ut_sb[:],
                    xall[:, t * C : (t + 1) * C],
                    start=True,
                    stop=True,
                )
                if j % 2 == 0:
                    nc.scalar.copy(yb[:, j * C : (j + 1) * C], bank[:])
                else:
                    nc.vector.tensor_copy(yb[:, j * C : (j + 1) * C], bank[:])
                if j % (KS // 2) == KS // 2 - 1:
                    h = j // (KS // 2)
                    ydst = y[g].rearrange("p (h c) -> p h c", h=2)[:, h, :]
                    nc.gpsimd.dma_start(
                        out=ydst,
                        in_=yb[:, h * (KS // 2) * C : (h + 1) * (KS // 2) * C],
                    )

        # software pipeline: carry chain runs one group ahead of UT stage
        carry_stage(0)
        for g in range(1, NG):
            carry_stage(g)
            ut_stage(g - 1)
        ut_stage(NG - 1)

    nc.compile()
    return nc


def _get_program():
    if "nc" not in _CACHE:
        _CACHE["nc"] = _build_program()
    return _CACHE["nc"]


def _shard(X):
    """(4, 8192, 32, 32) f32 -> 8 fp16 slabs swizzled to (g, p, ks, c)."""
    Xv = X.reshape(B, L, C_FULL)
    shards = []
    for i in range(N_CORES):
        b, h = i // 2, i % 2
        s = Xv[b, :, h * C : (h + 1) * C].astype(np.float16)    # (8192, 512)
        s = s.reshape(NG, KS, P, C).transpose(0, 2, 1, 3)       # (g, p, ks, c)
        shards.append(np.ascontiguousarray(s.reshape(NG, P, KS * C)))
    return shards


def _unshard(parts):
    out = np.empty((B, L, C_FULL), dtype=np.float32)
    for i in range(N_CORES):
        b, h = i // 2, i % 2
        p = parts[i].reshape(NG, P, KS, C).transpose(0, 2, 1, 3)  # (g, ks, p, c)
        out[b, :, h * C : (h + 1) * C] = p.reshape(L, C).astype(np.float32)
    return out.reshape(B, L, D, N)


def kernel(X_in, _trace=False, _tmpdir=None, _trace_cores=None):
    X = np.asarray(X_in, dtype=np.float32)
    assert X.shape == (B, L, D, N), X.shape
    nc = _get_program()
    ut, ohw, pmw = _host_consts()
    in_maps = [{"x": s, "ut": ut, "ohw": ohw, "pmw": pmw} for s in _shard(X)]
    kwargs = {}
    if _trace:
        kwargs = dict(
            trace=True,
            tmpdir=_tmpdir,
            trace_cores=_trace_cores or list(range(N_CORES)),
        )
    res = run_bass_kernel_spmd(nc, in_maps, core_ids=list(range(N_CORES)), **kwargs)
    out = _unshard([res.results[i]["y"] for i in range(N_CORES)])
    kernel.last_results = res
    return out
